# revision 39
# baseline (speedup 1.0000x reference)
"""AttentionPooling TRN2 kernel.

Math: for each batch b:
    scores = x_b @ W.T + bias            (N, ATT)
    logits = scores @ A.T                (N, M)   [as (M, N) transposed]
    weights = softmax(logits over N)
    out_b = weights @ x_b                (M, C)

Two exact algebraic simplifications:
  * logits = x @ (A @ W).T + (A @ bias); the (A @ bias)[m] term is constant
    over N, so softmax cancels it -> bias drops out entirely.
  * With G = A @ W (M, C) precomputed on-device (tiny), the big scores
    matmul (B*N*C*ATT flops) collapses into logits = x @ G.T (B*N*C*M).

Softmax is computed without the max-subtraction: |logits| <~ 40 here, so
exp() stays well inside fp32 range, and softmax(z) == softmax(z - max)
exactly in infinite precision.

Sharding: data-parallel over B across the 8 cores (one batch each), no
collectives. Per core:
  - load x chunk [512, 1024] (natural layout, rhs of pooling matmul)
  - PE-transpose to xT [C-tiles, n] (rhs of logits matmul)
  - logits^T [64, 512] = G^T-tiles^T @ xT-tiles   (K = C)
  - E = exp(logits^T) on ACT; per-chunk row-sums on DVE
  - E^T via PE transpose (lhsT of pooling matmul)
  - pooling accumulate psum[64, 1024] += E^T-tile^T @ x-tile  (K = n)
  - after all chunks: scale rows by 1/sum, DMA out.
"""

import numpy as np

import concourse.bacc as bacc
import concourse.mybir as mybir
import concourse.tile as tile
from concourse.bass_utils import run_bass_kernel_spmd

B, N, C = 8, 4096, 1024
ATT, M = 512, 64
NCORES = 8
CHUNK = 512
NCHUNKS = N // CHUNK  # 8
SUB = CHUNK // 128  # 4 n-subtiles per chunk
CT = C // 128  # 8 c-tiles

F32 = mybir.dt.float32
# Wide-matmul dtype: f32r = fp32 rounded to 11 mantissa bits (walrus
# fp32_to_fp32r keeps s+8e+11m, zeroing the low 12 bits). PE streams f32r
# 1 row/cycle at free-dim >= 256 (vs 4 cycles/row for fp32) and transposes
# at 1.5 (vs 2). Rounding error ~2.4e-4 against the 2e-2 gate.
DT = mybir.dt.float32r
R = mybir.dt.float32r
# (A bf16 x-shadow for transposes/logits was tried and reverted: gpsimd CAST
# runs at ~36 G elem/s and paced the kernel at 144us, and bf16 logits put
# max-rel error at 1.9e-2 -- at the 2e-2 gate. f32r keeps 1.4e-3.)
# fp16 keeps the same 11-bit mantissa class as f32r but is a 2-byte dtype:
# PE weight loads for the x-transposes run at 1 cycle/row instead of ~1.6,
# and the psum->sbuf xT drains halve. x (|x|<~6 sigma) is far inside fp16
# range; products accumulate in fp32 PSUM.
H = mybir.dt.float16

Exp = mybir.ActivationFunctionType.Exp
AX = mybir.AxisListType
ALU = mybir.AluOpType


def build_nc():
    nc = bacc.Bacc("TRN2", target_bir_lowering=False, debug=False)

    x_d = nc.dram_tensor("x", [N, C], DT, kind="ExternalInput")
    w_d = nc.dram_tensor("w", [ATT, C], DT, kind="ExternalInput")
    at_d = nc.dram_tensor("at", [ATT, M], DT, kind="ExternalInput")
    id_d = nc.dram_tensor("ident", [128, 128], DT, kind="ExternalInput")
    o_d = nc.dram_tensor("o", [M, C], F32, kind="ExternalOutput")

    with tile.TileContext(nc) as tc:
        with (
            tc.tile_pool(name="const", bufs=1) as constp,
            tc.tile_pool(name="xpool", bufs=24) as xpool,
            tc.tile_pool(name="xhpool", bufs=12) as xhpool,
            tc.tile_pool(name="xtp", bufs=2) as xtp,
            tc.tile_pool(name="small", bufs=2) as smallp,
            tc.tile_pool(name="outp", bufs=1) as outp,
            tc.tile_pool(name="psT", bufs=3, space="PSUM") as psT,
            tc.tile_pool(name="psL", bufs=2, space="PSUM") as psL,
            tc.tile_pool(name="psE", bufs=1, space="PSUM") as psE,
            tc.tile_pool(name="psO", bufs=1, space="PSUM") as psO,
        ):
            # chunk row counts: short first chunk so the PE transpose stream
            # starts as soon as 1MB has landed; short last chunk to shorten
            # the end-of-kernel dependency tail. 256-row logits matmuls still
            # hit the fast f32r path (free dim >= 256).
            SIZES = [256] + [512] * 7 + [256]
            ROW0 = [sum(SIZES[:k]) for k in range(len(SIZES))]
            NCH = len(SIZES)

            # issue the first x chunk ahead of the const loads so the PE's
            # transpose stream starts as early as possible
            def load_chunk(k):
                tiles = []
                for i in range(SIZES[k] // 128):
                    xt_ = xpool.tile([128, C], DT, tag="x", name=f"x_{k}_{i}")
                    r0 = ROW0[k] + i * 128
                    nc.sync.dma_start(xt_[:], x_d.ap()[r0 : r0 + 128, :])
                    tiles.append(xt_)
                return tiles

            PREFETCH = 5

            id_sb = constp.tile([128, 128], DT)
            nc.sync.dma_start(id_sb[:], id_d.ap())
            pending = [load_chunk(0)]
            id_hf = constp.tile([128, 128], H, name="id_hf")
            nc.vector.tensor_copy(id_hf[:], id_sb[:])

            # fp16 shadow of the popped chunk, split DVE/ACT. Emitted at pop
            # time: the chunk's DMA completed PREFETCH chunks ago, so the
            # in-order vector/scalar queues never stall on it, and transpose
            # (j, i) only waits for tile i's convert (~0.5us, hidden under
            # the previous chunk's logits matmul).
            def convert_chunk(k, x_tiles):
                tiles = []
                for i, xt_ in enumerate(x_tiles):
                    xh = xhpool.tile([128, C], H, tag="xh", name=f"xh_{k}_{i}")
                    if i % 4 == 3:
                        nc.scalar.copy(xh[:], xt_[:])
                    else:
                        nc.vector.tensor_copy(xh[:], xt_[:])
                    tiles.append(xh)
                return tiles
            at_sb = constp.tile([128, ATT // 128, M], DT)
            nc.sync.dma_start(
                at_sb[:], at_d.ap().rearrange("(t p) m -> p t m", p=128)
            )
            # W in two half-C loads so G's first psum half can start sooner
            w_half = []
            for h in range(2):
                wh = constp.tile([128, ATT // 128, 512], DT, name=f"w_sb{h}")
                nc.sync.dma_start(
                    wh[:],
                    w_d.ap().rearrange("(t p) c -> p t c", p=128)[
                        :, :, 512 * h : 512 * (h + 1)
                    ],
                )
                w_half.append(wh)
            for k in range(1, PREFETCH + 1):
                pending.append(load_chunk(k))

            # HAM warm-up on a memset tile (no DMA dependency -- the PE starts
            # the moment the DVE memset lands). HEAVY f32r 512-wide streams,
            # the same activity class as the real work: the HAM's ~24us
            # half-duty (k=4/8) probation window is triggered by the first
            # sustained heavy activity, so trip it at t~0 -- it then elapses
            # during the DMA-limited ramp-in instead of throttling the
            # mid-kernel transpose/matmul pipeline (measured: 129ns ->
            # 87ns per transpose once k=8/8 is granted).
            warm_f32 = constp.tile([128, 512], F32, name="warm_f32")
            nc.vector.memset(warm_f32[:], 0.0)
            # memset cannot emit f32r directly (ISA memset_set_value_type);
            # a DVE copy is a valid f32r-rounding producer
            warm_in = constp.tile([128, 512], DT, name="warm_in")
            nc.vector.tensor_copy(warm_in[:], warm_f32[:])
            warm_ps = psT.tile([128, 512], F32, tag="pst", name="warm_ps")
            for r in range(16):
                nc.tensor.matmul(
                    warm_ps[:64, :], warm_in[:, :64], warm_in[:],
                    start=(r == 0), stop=(r == 15),
                )
            warm_out = constp.tile([64, 512], F32, name="warm_out")
            nc.vector.tensor_copy(warm_out[:], warm_ps[:64, :])

            gT_sb = constp.tile([128, CT * M], H)

            def emit_g():
                # G natural [64, C] = A^T-tiles^T @ W-tiles (two 512-wide psum
                # halves), then PE-transpose into gT [C-tiles, 64].
                psg = [psL.tile([M, 512], F32, tag="psl", name=f"psg_{h}")
                       for h in range(2)]
                for h in range(2):
                    for t in range(ATT // 128):
                        nc.tensor.matmul(
                            psg[h][:],
                            at_sb[:, t, :].bitcast(R),
                            w_half[h][:, t, :].bitcast(R),
                            start=(t == 0),
                            stop=(t == ATT // 128 - 1),
                        )
                g_sb = constp.tile([M, C], H)
                for h in range(2):
                    nc.vector.tensor_copy(g_sb[:, 512 * h : 512 * (h + 1)], psg[h][:])
                psgt = psT.tile([128, CT * M], H, tag="pst", name="psgt")
                for j in range(CT):
                    nc.tensor.transpose(
                        psgt[:, M * j : M * (j + 1)],
                        g_sb[:, 128 * j : 128 * (j + 1)],
                        id_hf[:M, :M],
                    )
                nc.scalar.copy(gT_sb[:], psgt[:])

            sums_sb = outp.tile([M, NCH], F32)
            # one accumulator tile per PSUM bank -- a [64, 1024] tensor would
            # span two banks and bank-crossing APs are not HW-safe
            psOut = [psO.tile([M, 512], F32, name=f"psOut_{h}") for h in range(C // 512)]

            def chunk_tail(k, e_sb, x_tiles):
                # E^T via PE transpose (PE waits on ACT exp, which overlaps
                # the next chunk's x-transposes), then pooling accumulate.
                sub = len(x_tiles)
                pse = psE.tile([128, sub * M], DT, tag="pse", name=f"pse_{k}")
                for i in range(sub):
                    nc.tensor.transpose(
                        pse[:, M * i : M * (i + 1)].bitcast(R),
                        e_sb[:, 128 * i : 128 * (i + 1)].bitcast(R),
                        id_sb[:M, :M].bitcast(R),
                    )
                eT_sb = smallp.tile([128, sub * M], DT, tag="et", name=f"eT_{k}")
                nc.scalar.copy(eT_sb[:], pse[:])
                for i in range(sub):
                    for h in range(C // 512):
                        nc.tensor.matmul(
                            psOut[h][:],
                            eT_sb[:, M * i : M * (i + 1)].bitcast(R),
                            x_tiles[i][:, 512 * h : 512 * (h + 1)].bitcast(R),
                            start=(k == 0 and i == 0),
                            stop=(k == NCH - 1 and i == sub - 1),
                        )

            prev = None
            for k in range(NCH):
                x_tiles = pending.pop(0)
                xh_tiles = convert_chunk(k, x_tiles)
                if k + PREFETCH + 1 < NCH:
                    pending.append(load_chunk(k + PREFETCH + 1))
                nrows = SIZES[k]
                sub = nrows // 128

                xT = xtp.tile([128, CT * nrows], H, tag="xt", name=f"xT_{k}")
                for j in range(CT):
                    pst = psT.tile([128, nrows], H, tag="pst", name=f"pst_{k}_{j}")
                    for i in range(sub):
                        nc.tensor.transpose(
                            pst[:, 128 * i : 128 * (i + 1)],
                            xh_tiles[i][:, 128 * j : 128 * (j + 1)],
                            id_hf[:],
                        )
                    # split the PSUM drains between DVE and the scalar engine
                    # (gpsimd/Pool cannot access PSUM)
                    if j % 2 == 0:
                        nc.vector.tensor_copy(xT[:, nrows * j : nrows * (j + 1)], pst[:])
                    else:
                        nc.scalar.copy(xT[:, nrows * j : nrows * (j + 1)], pst[:])

                if k == 0:
                    emit_g()
                if prev is not None:
                    chunk_tail(*prev)

                psl = psL.tile([M, nrows], F32, tag="psl", name=f"psl_{k}")
                for j in range(CT):
                    nc.tensor.matmul(
                        psl[:],
                        gT_sb[:, M * j : M * (j + 1)],
                        xT[:, nrows * j : nrows * (j + 1)],
                        start=(j == 0),
                        stop=(j == CT - 1),
                    )

                e_sb = smallp.tile([M, nrows], DT, tag="e", name=f"e_{k}")
                nc.scalar.activation(e_sb[:], psl[:], Exp)
                nc.vector.tensor_reduce(
                    sums_sb[:, k : k + 1], e_sb[:], axis=AX.X, op=ALU.add
                )

                prev = (k, e_sb, x_tiles)

            chunk_tail(*prev)

            total = outp.tile([M, 1], F32)
            nc.vector.tensor_reduce(total[:], sums_sb[:], axis=AX.X, op=ALU.add)
            recip = outp.tile([M, 1], F32)
            nc.vector.reciprocal(recip[:], total[:])
            out_sb = outp.tile([M, C], F32)
            for h in range(C // 512):
                nc.vector.tensor_scalar_mul(
                    out_sb[:, 512 * h : 512 * (h + 1)], psOut[h][:], recip[:]
                )
            nc.sync.dma_start(o_d.ap(), out_sb[:])

    nc.compile()
    return nc


_CACHE = {}


def _get_nc():
    if "nc" not in _CACHE:
        _CACHE["nc"] = build_nc()
    return _CACHE["nc"]


def _in_maps(x, W, attention_vectors):
    at = np.ascontiguousarray(attention_vectors.T).astype(np.float32, copy=False)
    ident = np.eye(128, dtype=np.float32)
    W = np.ascontiguousarray(W).astype(np.float32, copy=False)
    return [
        {
            "x": np.ascontiguousarray(x[i]).astype(np.float32, copy=False),
            "w": W,
            "at": at,
            "ident": ident,
        }
        for i in range(x.shape[0])
    ]


def _run(x, W, attention_vectors, **spmd_kwargs):
    nc = _get_nc()
    return run_bass_kernel_spmd(
        nc, _in_maps(x, W, attention_vectors), core_ids=list(range(NCORES)),
        **spmd_kwargs,
    )


def kernel(x, W, b, attention_vectors):
    del b  # softmax over N cancels the (A @ b)[m] logit offset exactly
    x = np.asarray(x, dtype=np.float32)
    br = _run(x, np.asarray(W), np.asarray(attention_vectors))
    return np.stack([r["o"] for r in br.results], axis=0)



# revision 41
# speedup vs baseline: 1.0412x; 1.0412x over previous
"""AttentionPooling TRN2 kernel.

Math: for each batch b:
    scores = x_b @ W.T + bias            (N, ATT)
    logits = scores @ A.T                (N, M)   [as (M, N) transposed]
    weights = softmax(logits over N)
    out_b = weights @ x_b                (M, C)

Two exact algebraic simplifications:
  * logits = x @ (A @ W).T + (A @ bias); the (A @ bias)[m] term is constant
    over N, so softmax cancels it -> bias drops out entirely.
  * With G = A @ W (M, C) precomputed on-device (tiny), the big scores
    matmul (B*N*C*ATT flops) collapses into logits = x @ G.T (B*N*C*M).

Softmax is computed without the max-subtraction: |logits| <~ 40 here, so
exp() stays well inside fp32 range, and softmax(z) == softmax(z - max)
exactly in infinite precision.

Sharding: data-parallel over B across the 8 cores (one batch each), no
collectives. Per core:
  - load x chunk [512, 1024] (natural layout, rhs of pooling matmul)
  - PE-transpose to xT [C-tiles, n] (rhs of logits matmul)
  - logits^T [64, 512] = G^T-tiles^T @ xT-tiles   (K = C)
  - E = exp(logits^T) on ACT; per-chunk row-sums on DVE
  - E^T via PE transpose (lhsT of pooling matmul)
  - pooling accumulate psum[64, 1024] += E^T-tile^T @ x-tile  (K = n)
  - after all chunks: scale rows by 1/sum, DMA out.
"""

import numpy as np

import concourse.bacc as bacc
import concourse.mybir as mybir
import concourse.tile as tile
from concourse.bass_utils import run_bass_kernel_spmd

B, N, C = 8, 4096, 1024
ATT, M = 512, 64
NCORES = 8
CHUNK = 512
NCHUNKS = N // CHUNK  # 8
SUB = CHUNK // 128  # 4 n-subtiles per chunk
CT = C // 128  # 8 c-tiles

F32 = mybir.dt.float32
# Wide-matmul dtype: f32r = fp32 rounded to 11 mantissa bits (walrus
# fp32_to_fp32r keeps s+8e+11m, zeroing the low 12 bits). PE streams f32r
# 1 row/cycle at free-dim >= 256 (vs 4 cycles/row for fp32) and transposes
# at 1.5 (vs 2). Rounding error ~2.4e-4 against the 2e-2 gate.
DT = mybir.dt.float32r
R = mybir.dt.float32r
# (A bf16 x-shadow for transposes/logits was tried and reverted: gpsimd CAST
# runs at ~36 G elem/s and paced the kernel at 144us, and bf16 logits put
# max-rel error at 1.9e-2 -- at the 2e-2 gate. f32r keeps 1.4e-3.)
# fp16 keeps the same 11-bit mantissa class as f32r but is a 2-byte dtype:
# PE weight loads for the x-transposes run at 1 cycle/row instead of ~1.6,
# and the psum->sbuf xT drains halve. x (|x|<~6 sigma) is far inside fp16
# range; products accumulate in fp32 PSUM.
H = mybir.dt.float16

Exp = mybir.ActivationFunctionType.Exp
AX = mybir.AxisListType
ALU = mybir.AluOpType


def build_nc():
    nc = bacc.Bacc("TRN2", target_bir_lowering=False, debug=False)

    x_d = nc.dram_tensor("x", [N, C], DT, kind="ExternalInput")
    w_d = nc.dram_tensor("w", [ATT, C], DT, kind="ExternalInput")
    at_d = nc.dram_tensor("at", [ATT, M], DT, kind="ExternalInput")
    id_d = nc.dram_tensor("ident", [128, 128], DT, kind="ExternalInput")
    o_d = nc.dram_tensor("o", [M, C], F32, kind="ExternalOutput")

    with tile.TileContext(nc) as tc:
        with (
            tc.tile_pool(name="const", bufs=1) as constp,
            tc.tile_pool(name="xpool", bufs=24) as xpool,
            tc.tile_pool(name="xhpool", bufs=12) as xhpool,
            tc.tile_pool(name="xtp", bufs=2) as xtp,
            tc.tile_pool(name="small", bufs=2) as smallp,
            tc.tile_pool(name="outp", bufs=1) as outp,
            tc.tile_pool(name="psT", bufs=3, space="PSUM") as psT,
            tc.tile_pool(name="psL", bufs=2, space="PSUM") as psL,
            tc.tile_pool(name="psE", bufs=1, space="PSUM") as psE,
            tc.tile_pool(name="psO", bufs=1, space="PSUM") as psO,
        ):
            # chunk row counts: short first chunk so the PE transpose stream
            # starts as soon as 1MB has landed; short last chunk to shorten
            # the end-of-kernel dependency tail. 256-row logits matmuls still
            # hit the fast f32r path (free dim >= 256).
            SIZES = [256] + [512] * 7 + [256]
            ROW0 = [sum(SIZES[:k]) for k in range(len(SIZES))]
            NCH = len(SIZES)

            # issue the first x chunk ahead of the const loads so the PE's
            # transpose stream starts as early as possible
            def load_chunk(k):
                tiles = []
                for i in range(SIZES[k] // 128):
                    xt_ = xpool.tile([128, C], DT, tag="x", name=f"x_{k}_{i}")
                    r0 = ROW0[k] + i * 128
                    nc.sync.dma_start(xt_[:], x_d.ap()[r0 : r0 + 128, :])
                    tiles.append(xt_)
                return tiles

            PREFETCH = 5

            id_sb = constp.tile([128, 128], DT)
            nc.sync.dma_start(id_sb[:], id_d.ap())
            pending = [load_chunk(0)]
            id_hf = constp.tile([128, 128], H, name="id_hf")
            nc.vector.tensor_copy(id_hf[:], id_sb[:])

            # fp16 shadow of a chunk, emitted one chunk AHEAD of its use so
            # the converts run under the previous chunk's PE work instead of
            # stalling the transposes. Each tile converts as two half-tiles
            # on DVE (c 0:512, feeds transpose groups j=0..3) and ACT
            # (c 512:, groups j=4..7), so group j=0 waits only ~270ns.
            def convert_chunk(k, x_tiles):
                tiles = []
                for i, xt_ in enumerate(x_tiles):
                    xh = xhpool.tile([128, C], H, tag="xh", name=f"xh_{k}_{i}")
                    nc.vector.tensor_copy(xh[:, :512], xt_[:, :512])
                    nc.scalar.copy(xh[:, 512:], xt_[:, 512:])
                    tiles.append(xh)
                return tiles
            at_sb = constp.tile([128, ATT // 128, M], DT)
            nc.sync.dma_start(
                at_sb[:], at_d.ap().rearrange("(t p) m -> p t m", p=128)
            )
            # W in two half-C loads so G's first psum half can start sooner
            w_half = []
            for h in range(2):
                wh = constp.tile([128, ATT // 128, 512], DT, name=f"w_sb{h}")
                nc.sync.dma_start(
                    wh[:],
                    w_d.ap().rearrange("(t p) c -> p t c", p=128)[
                        :, :, 512 * h : 512 * (h + 1)
                    ],
                )
                w_half.append(wh)
            for k in range(1, PREFETCH + 1):
                pending.append(load_chunk(k))

            # HAM warm-up on a memset tile (no DMA dependency -- the PE starts
            # the moment the DVE memset lands). HEAVY f32r 512-wide streams,
            # the same activity class as the real work: the HAM's ~24us
            # half-duty (k=4/8) probation window is triggered by the first
            # sustained heavy activity, so trip it at t~0 -- it then elapses
            # during the DMA-limited ramp-in instead of throttling the
            # mid-kernel transpose/matmul pipeline (measured: 129ns ->
            # 87ns per transpose once k=8/8 is granted).
            warm_f32 = constp.tile([128, 512], F32, name="warm_f32")
            nc.vector.memset(warm_f32[:], 0.0)
            # memset cannot emit f32r directly (ISA memset_set_value_type);
            # a DVE copy is a valid f32r-rounding producer
            warm_in = constp.tile([128, 512], DT, name="warm_in")
            nc.vector.tensor_copy(warm_in[:], warm_f32[:])
            warm_ps = psT.tile([128, 512], F32, tag="pst", name="warm_ps")
            for r in range(16):
                nc.tensor.matmul(
                    warm_ps[:64, :], warm_in[:, :64], warm_in[:],
                    start=(r == 0), stop=(r == 15),
                )
            warm_out = constp.tile([64, 512], F32, name="warm_out")
            nc.vector.tensor_copy(warm_out[:], warm_ps[:64, :])

            gT_sb = constp.tile([128, CT * M], H)

            def emit_g():
                # G natural [64, C] = A^T-tiles^T @ W-tiles (two 512-wide psum
                # halves), then PE-transpose into gT [C-tiles, 64].
                psg = [psL.tile([M, 512], F32, tag="psl", name=f"psg_{h}")
                       for h in range(2)]
                for h in range(2):
                    for t in range(ATT // 128):
                        nc.tensor.matmul(
                            psg[h][:],
                            at_sb[:, t, :].bitcast(R),
                            w_half[h][:, t, :].bitcast(R),
                            start=(t == 0),
                            stop=(t == ATT // 128 - 1),
                        )
                g_sb = constp.tile([M, C], H)
                for h in range(2):
                    nc.vector.tensor_copy(g_sb[:, 512 * h : 512 * (h + 1)], psg[h][:])
                psgt = psT.tile([128, CT * M], H, tag="pst", name="psgt")
                for j in range(CT):
                    nc.tensor.transpose(
                        psgt[:, M * j : M * (j + 1)],
                        g_sb[:, 128 * j : 128 * (j + 1)],
                        id_hf[:M, :M],
                    )
                nc.scalar.copy(gT_sb[:], psgt[:])

            sums_sb = outp.tile([M, NCH], F32)
            # one accumulator tile per PSUM bank -- a [64, 1024] tensor would
            # span two banks and bank-crossing APs are not HW-safe
            psOut = [psO.tile([M, 512], F32, name=f"psOut_{h}") for h in range(C // 512)]

            def chunk_tail(k, e_sb, x_tiles):
                # E^T via PE transpose (PE waits on ACT exp, which overlaps
                # the next chunk's x-transposes), then pooling accumulate.
                sub = len(x_tiles)
                pse = psE.tile([128, sub * M], DT, tag="pse", name=f"pse_{k}")
                for i in range(sub):
                    nc.tensor.transpose(
                        pse[:, M * i : M * (i + 1)].bitcast(R),
                        e_sb[:, 128 * i : 128 * (i + 1)].bitcast(R),
                        id_sb[:M, :M].bitcast(R),
                    )
                eT_sb = smallp.tile([128, sub * M], DT, tag="et", name=f"eT_{k}")
                nc.scalar.copy(eT_sb[:], pse[:])
                for i in range(sub):
                    for h in range(C // 512):
                        nc.tensor.matmul(
                            psOut[h][:],
                            eT_sb[:, M * i : M * (i + 1)].bitcast(R),
                            x_tiles[i][:, 512 * h : 512 * (h + 1)].bitcast(R),
                            start=(k == 0 and i == 0),
                            stop=(k == NCH - 1 and i == sub - 1),
                        )

            pending_h = [convert_chunk(0, pending[0])]
            prev = None
            for k in range(NCH):
                x_tiles = pending.pop(0)
                xh_tiles = pending_h.pop(0)
                if k + 1 < NCH:
                    pending_h.append(convert_chunk(k + 1, pending[0]))
                if k + PREFETCH + 1 < NCH:
                    pending.append(load_chunk(k + PREFETCH + 1))
                nrows = SIZES[k]
                sub = nrows // 128

                xT = xtp.tile([128, CT * nrows], H, tag="xt", name=f"xT_{k}")
                for j in range(CT):
                    pst = psT.tile([128, nrows], H, tag="pst", name=f"pst_{k}_{j}")
                    for i in range(sub):
                        nc.tensor.transpose(
                            pst[:, 128 * i : 128 * (i + 1)],
                            xh_tiles[i][:, 128 * j : 128 * (j + 1)],
                            id_hf[:],
                        )
                    # split the PSUM drains between DVE and the scalar engine
                    # (gpsimd/Pool cannot access PSUM)
                    if j % 2 == 0:
                        nc.vector.tensor_copy(xT[:, nrows * j : nrows * (j + 1)], pst[:])
                    else:
                        nc.scalar.copy(xT[:, nrows * j : nrows * (j + 1)], pst[:])

                if k == 0:
                    emit_g()
                if prev is not None:
                    chunk_tail(*prev)

                psl = psL.tile([M, nrows], F32, tag="psl", name=f"psl_{k}")
                for j in range(CT):
                    nc.tensor.matmul(
                        psl[:],
                        gT_sb[:, M * j : M * (j + 1)],
                        xT[:, nrows * j : nrows * (j + 1)],
                        start=(j == 0),
                        stop=(j == CT - 1),
                    )

                e_sb = smallp.tile([M, nrows], DT, tag="e", name=f"e_{k}")
                nc.scalar.activation(e_sb[:], psl[:], Exp)
                nc.vector.tensor_reduce(
                    sums_sb[:, k : k + 1], e_sb[:], axis=AX.X, op=ALU.add
                )

                prev = (k, e_sb, x_tiles)

            chunk_tail(*prev)

            total = outp.tile([M, 1], F32)
            nc.vector.tensor_reduce(total[:], sums_sb[:], axis=AX.X, op=ALU.add)
            recip = outp.tile([M, 1], F32)
            nc.vector.reciprocal(recip[:], total[:])
            out_sb = outp.tile([M, C], F32)
            for h in range(C // 512):
                nc.vector.tensor_scalar_mul(
                    out_sb[:, 512 * h : 512 * (h + 1)], psOut[h][:], recip[:]
                )
            nc.sync.dma_start(o_d.ap(), out_sb[:])

    nc.compile()
    return nc


_CACHE = {}


def _get_nc():
    if "nc" not in _CACHE:
        _CACHE["nc"] = build_nc()
    return _CACHE["nc"]


def _in_maps(x, W, attention_vectors):
    at = np.ascontiguousarray(attention_vectors.T).astype(np.float32, copy=False)
    ident = np.eye(128, dtype=np.float32)
    W = np.ascontiguousarray(W).astype(np.float32, copy=False)
    return [
        {
            "x": np.ascontiguousarray(x[i]).astype(np.float32, copy=False),
            "w": W,
            "at": at,
            "ident": ident,
        }
        for i in range(x.shape[0])
    ]


def _run(x, W, attention_vectors, **spmd_kwargs):
    nc = _get_nc()
    return run_bass_kernel_spmd(
        nc, _in_maps(x, W, attention_vectors), core_ids=list(range(NCORES)),
        **spmd_kwargs,
    )


def kernel(x, W, b, attention_vectors):
    del b  # softmax over N cancels the (A @ b)[m] logit offset exactly
    x = np.asarray(x, dtype=np.float32)
    br = _run(x, np.asarray(W), np.asarray(attention_vectors))
    return np.stack([r["o"] for r in br.results], axis=0)



# revision 43
# speedup vs baseline: 1.0817x; 1.0389x over previous
"""AttentionPooling TRN2 kernel.

Math: for each batch b:
    scores = x_b @ W.T + bias            (N, ATT)
    logits = scores @ A.T                (N, M)   [as (M, N) transposed]
    weights = softmax(logits over N)
    out_b = weights @ x_b                (M, C)

Two exact algebraic simplifications:
  * logits = x @ (A @ W).T + (A @ bias); the (A @ bias)[m] term is constant
    over N, so softmax cancels it -> bias drops out entirely.
  * With G = A @ W (M, C) precomputed on-device (tiny), the big scores
    matmul (B*N*C*ATT flops) collapses into logits = x @ G.T (B*N*C*M).

Softmax is computed without the max-subtraction: |logits| <~ 40 here, so
exp() stays well inside fp32 range, and softmax(z) == softmax(z - max)
exactly in infinite precision.

Sharding: data-parallel over B across the 8 cores (one batch each), no
collectives. Per core:
  - load x chunk [512, 1024] (natural layout, rhs of pooling matmul)
  - PE-transpose to xT [C-tiles, n] (rhs of logits matmul)
  - logits^T [64, 512] = G^T-tiles^T @ xT-tiles   (K = C)
  - E = exp(logits^T) on ACT; per-chunk row-sums on DVE
  - E^T via PE transpose (lhsT of pooling matmul)
  - pooling accumulate psum[64, 1024] += E^T-tile^T @ x-tile  (K = n)
  - after all chunks: scale rows by 1/sum, DMA out.
"""

import numpy as np

import concourse.bacc as bacc
import concourse.mybir as mybir
import concourse.tile as tile
from concourse.bass_utils import run_bass_kernel_spmd

B, N, C = 8, 4096, 1024
ATT, M = 512, 64
NCORES = 8
CHUNK = 512
NCHUNKS = N // CHUNK  # 8
SUB = CHUNK // 128  # 4 n-subtiles per chunk
CT = C // 128  # 8 c-tiles

F32 = mybir.dt.float32
# Wide-matmul dtype: f32r = fp32 rounded to 11 mantissa bits (walrus
# fp32_to_fp32r keeps s+8e+11m, zeroing the low 12 bits). PE streams f32r
# 1 row/cycle at free-dim >= 256 (vs 4 cycles/row for fp32) and transposes
# at 1.5 (vs 2). Rounding error ~2.4e-4 against the 2e-2 gate.
DT = mybir.dt.float32r
R = mybir.dt.float32r
# (A bf16 x-shadow for transposes/logits was tried and reverted: gpsimd CAST
# runs at ~36 G elem/s and paced the kernel at 144us, and bf16 logits put
# max-rel error at 1.9e-2 -- at the 2e-2 gate. f32r keeps 1.4e-3.)
# fp16 keeps the same 11-bit mantissa class as f32r but is a 2-byte dtype:
# PE weight loads for the x-transposes run at 1 cycle/row instead of ~1.6,
# and the psum->sbuf xT drains halve. x (|x|<~6 sigma) is far inside fp16
# range; products accumulate in fp32 PSUM.
H = mybir.dt.float16

Exp = mybir.ActivationFunctionType.Exp
AX = mybir.AxisListType
ALU = mybir.AluOpType


def build_nc():
    nc = bacc.Bacc("TRN2", target_bir_lowering=False, debug=False)

    x_d = nc.dram_tensor("x", [N, C], DT, kind="ExternalInput")
    w_d = nc.dram_tensor("w", [ATT, C], DT, kind="ExternalInput")
    at_d = nc.dram_tensor("at", [ATT, M], DT, kind="ExternalInput")
    id_d = nc.dram_tensor("ident", [128, 128], DT, kind="ExternalInput")
    o_d = nc.dram_tensor("o", [M, C], F32, kind="ExternalOutput")

    with tile.TileContext(nc) as tc:
        with (
            tc.tile_pool(name="const", bufs=1) as constp,
            tc.tile_pool(name="xpool", bufs=24) as xpool,
            tc.tile_pool(name="xhpool", bufs=12) as xhpool,
            tc.tile_pool(name="xtp", bufs=2) as xtp,
            tc.tile_pool(name="small", bufs=2) as smallp,
            tc.tile_pool(name="outp", bufs=1) as outp,
            tc.tile_pool(name="psT", bufs=3, space="PSUM") as psT,
            tc.tile_pool(name="psL", bufs=2, space="PSUM") as psL,
            tc.tile_pool(name="psE", bufs=1, space="PSUM") as psE,
            tc.tile_pool(name="psO", bufs=1, space="PSUM") as psO,
        ):
            # chunk row counts: short first chunk so the PE transpose stream
            # starts as soon as 1MB has landed; short last chunk to shorten
            # the end-of-kernel dependency tail. 256-row logits matmuls still
            # hit the fast f32r path (free dim >= 256).
            SIZES = [256] + [512] * 7 + [256]
            ROW0 = [sum(SIZES[:k]) for k in range(len(SIZES))]
            NCH = len(SIZES)

            # issue the first x chunk ahead of the const loads so the PE's
            # transpose stream starts as early as possible
            def load_chunk(k):
                tiles = []
                for i in range(SIZES[k] // 128):
                    xt_ = xpool.tile([128, C], DT, tag="x", name=f"x_{k}_{i}")
                    r0 = ROW0[k] + i * 128
                    nc.sync.dma_start(xt_[:], x_d.ap()[r0 : r0 + 128, :])
                    tiles.append(xt_)
                return tiles

            PREFETCH = 5

            id_sb = constp.tile([128, 128], DT)
            nc.sync.dma_start(id_sb[:], id_d.ap())
            pending = [load_chunk(0)]
            id_hf = constp.tile([128, 128], H, name="id_hf")
            nc.vector.tensor_copy(id_hf[:], id_sb[:])

            # fp16 shadow of a chunk, emitted one chunk AHEAD of its use so
            # the converts run under the previous chunk's PE work instead of
            # stalling the transposes. Each tile converts as two half-tiles
            # on DVE (c 0:512, feeds transpose groups j=0..3) and ACT
            # (c 512:, groups j=4..7), so group j=0 waits only ~270ns.
            def convert_chunk(k, x_tiles):
                tiles = []
                for i, xt_ in enumerate(x_tiles):
                    xh = xhpool.tile([128, C], H, tag="xh", name=f"xh_{k}_{i}")
                    nc.vector.tensor_copy(xh[:, :512], xt_[:, :512])
                    nc.scalar.copy(xh[:, 512:], xt_[:, 512:])
                    tiles.append(xh)
                return tiles
            at_sb = constp.tile([128, ATT // 128, M], DT)
            nc.sync.dma_start(
                at_sb[:], at_d.ap().rearrange("(t p) m -> p t m", p=128)
            )
            # W in two half-C loads so G's first psum half can start sooner
            w_half = []
            for h in range(2):
                wh = constp.tile([128, ATT // 128, 512], DT, name=f"w_sb{h}")
                nc.sync.dma_start(
                    wh[:],
                    w_d.ap().rearrange("(t p) c -> p t c", p=128)[
                        :, :, 512 * h : 512 * (h + 1)
                    ],
                )
                w_half.append(wh)
            for k in range(1, PREFETCH + 1):
                pending.append(load_chunk(k))

            # HAM warm-up on a memset tile (no DMA dependency -- the PE starts
            # the moment the DVE memset lands). HEAVY f32r 512-wide streams,
            # the same activity class as the real work: the HAM's ~24us
            # half-duty (k=4/8) probation window is triggered by the first
            # sustained heavy activity, so trip it at t~0 -- it then elapses
            # during the DMA-limited ramp-in instead of throttling the
            # mid-kernel transpose/matmul pipeline (measured: 129ns ->
            # 87ns per transpose once k=8/8 is granted).
            warm_f32 = constp.tile([128, 512], F32, name="warm_f32")
            nc.vector.memset(warm_f32[:], 0.0)
            # memset cannot emit f32r directly (ISA memset_set_value_type);
            # a DVE copy is a valid f32r-rounding producer
            warm_in = constp.tile([128, 512], DT, name="warm_in")
            nc.vector.tensor_copy(warm_in[:], warm_f32[:])
            warm_ps = psT.tile([128, 512], F32, tag="pst", name="warm_ps")
            for r in range(16):
                nc.tensor.matmul(
                    warm_ps[:64, :], warm_in[:, :64], warm_in[:],
                    start=(r == 0), stop=(r == 15),
                )
            warm_out = constp.tile([64, 512], F32, name="warm_out")
            nc.vector.tensor_copy(warm_out[:], warm_ps[:64, :])

            gT_sb = constp.tile([128, CT * M], H)

            def emit_g():
                # G natural [64, C] = A^T-tiles^T @ W-tiles (two 512-wide psum
                # halves), then PE-transpose into gT [C-tiles, 64].
                psg = [psL.tile([M, 512], F32, tag="psl", name=f"psg_{h}")
                       for h in range(2)]
                for h in range(2):
                    for t in range(ATT // 128):
                        nc.tensor.matmul(
                            psg[h][:],
                            at_sb[:, t, :].bitcast(R),
                            w_half[h][:, t, :].bitcast(R),
                            start=(t == 0),
                            stop=(t == ATT // 128 - 1),
                        )
                g_sb = constp.tile([M, C], H)
                for h in range(2):
                    nc.vector.tensor_copy(g_sb[:, 512 * h : 512 * (h + 1)], psg[h][:])
                psgt = psT.tile([128, CT * M], H, tag="pst", name="psgt")
                for j in range(CT):
                    nc.tensor.transpose(
                        psgt[:, M * j : M * (j + 1)],
                        g_sb[:, 128 * j : 128 * (j + 1)],
                        id_hf[:M, :M],
                    )
                nc.scalar.copy(gT_sb[:], psgt[:])

            sums_sb = outp.tile([M, NCH], F32)
            # one accumulator tile per PSUM bank -- a [64, 1024] tensor would
            # span two banks and bank-crossing APs are not HW-safe
            psOut = [psO.tile([M, 512], F32, name=f"psOut_{h}") for h in range(C // 512)]

            def chunk_tail(k, e_sb, x_tiles):
                # E^T via PE transpose (PE waits on ACT exp, which overlaps
                # the next chunk's x-transposes), then pooling accumulate.
                sub = len(x_tiles)
                pse = psE.tile([128, sub * M], DT, tag="pse", name=f"pse_{k}")
                for i in range(sub):
                    nc.tensor.transpose(
                        pse[:, M * i : M * (i + 1)].bitcast(R),
                        e_sb[:, 128 * i : 128 * (i + 1)].bitcast(R),
                        id_sb[:M, :M].bitcast(R),
                    )
                eT_sb = smallp.tile([128, sub * M], DT, tag="et", name=f"eT_{k}")
                nc.scalar.copy(eT_sb[:], pse[:])
                for i in range(sub):
                    for h in range(C // 512):
                        nc.tensor.matmul(
                            psOut[h][:],
                            eT_sb[:, M * i : M * (i + 1)].bitcast(R),
                            x_tiles[i][:, 512 * h : 512 * (h + 1)].bitcast(R),
                            start=(k == 0 and i == 0),
                            stop=(k == NCH - 1 and i == sub - 1),
                        )

            pending_h = [convert_chunk(0, pending[0])]
            prev = None
            for k in range(NCH):
                x_tiles = pending.pop(0)
                xh_tiles = pending_h.pop(0)
                if k + PREFETCH + 1 < NCH:
                    pending.append(load_chunk(k + PREFETCH + 1))
                nrows = SIZES[k]
                sub = nrows // 128

                xT = xtp.tile([128, CT * nrows], H, tag="xt", name=f"xT_{k}")
                for j in range(CT):
                    pst = psT.tile([128, nrows], H, tag="pst", name=f"pst_{k}_{j}")
                    for i in range(sub):
                        nc.tensor.transpose(
                            pst[:, 128 * i : 128 * (i + 1)],
                            xh_tiles[i][:, 128 * j : 128 * (j + 1)],
                            id_hf[:],
                        )
                    # split the PSUM drains between DVE and the scalar engine
                    # (gpsimd/Pool cannot access PSUM)
                    if j % 2 == 0:
                        nc.vector.tensor_copy(xT[:, nrows * j : nrows * (j + 1)], pst[:])
                    else:
                        nc.scalar.copy(xT[:, nrows * j : nrows * (j + 1)], pst[:])

                # next chunk's fp16 converts queue AFTER this chunk's drains
                # (drains gate the logits matmul; converts are only needed at
                # the next chunk's transposes)
                if k + 1 < NCH:
                    pending_h.append(convert_chunk(k + 1, pending[0]))

                if k == 0:
                    emit_g()
                if prev is not None:
                    chunk_tail(*prev)

                psl = psL.tile([M, nrows], F32, tag="psl", name=f"psl_{k}")
                for j in range(CT):
                    nc.tensor.matmul(
                        psl[:],
                        gT_sb[:, M * j : M * (j + 1)],
                        xT[:, nrows * j : nrows * (j + 1)],
                        start=(j == 0),
                        stop=(j == CT - 1),
                    )

                e_sb = smallp.tile([M, nrows], DT, tag="e", name=f"e_{k}")
                nc.scalar.activation(e_sb[:], psl[:], Exp)
                nc.vector.tensor_reduce(
                    sums_sb[:, k : k + 1], e_sb[:], axis=AX.X, op=ALU.add
                )

                prev = (k, e_sb, x_tiles)

            chunk_tail(*prev)

            total = outp.tile([M, 1], F32)
            nc.vector.tensor_reduce(total[:], sums_sb[:], axis=AX.X, op=ALU.add)
            recip = outp.tile([M, 1], F32)
            nc.vector.reciprocal(recip[:], total[:])
            out_sb = outp.tile([M, C], F32)
            for h in range(C // 512):
                nc.vector.tensor_scalar_mul(
                    out_sb[:, 512 * h : 512 * (h + 1)], psOut[h][:], recip[:]
                )
            nc.sync.dma_start(o_d.ap(), out_sb[:])

    nc.compile()
    return nc


_CACHE = {}


def _get_nc():
    if "nc" not in _CACHE:
        _CACHE["nc"] = build_nc()
    return _CACHE["nc"]


def _in_maps(x, W, attention_vectors):
    at = np.ascontiguousarray(attention_vectors.T).astype(np.float32, copy=False)
    ident = np.eye(128, dtype=np.float32)
    W = np.ascontiguousarray(W).astype(np.float32, copy=False)
    return [
        {
            "x": np.ascontiguousarray(x[i]).astype(np.float32, copy=False),
            "w": W,
            "at": at,
            "ident": ident,
        }
        for i in range(x.shape[0])
    ]


def _run(x, W, attention_vectors, **spmd_kwargs):
    nc = _get_nc()
    return run_bass_kernel_spmd(
        nc, _in_maps(x, W, attention_vectors), core_ids=list(range(NCORES)),
        **spmd_kwargs,
    )


def kernel(x, W, b, attention_vectors):
    del b  # softmax over N cancels the (A @ b)[m] logit offset exactly
    x = np.asarray(x, dtype=np.float32)
    br = _run(x, np.asarray(W), np.asarray(attention_vectors))
    return np.stack([r["o"] for r in br.results], axis=0)

